# revision 25
# baseline (speedup 1.0000x reference)
"""Trainium2 Bass kernel for DiscriminativeLoss (segment_reduce).

Full inputs: embedding [8, 32, 65536] f32, seg_gt [8, 65536] i32 (labels 0..20,
0 = background).  Output: (var_loss, dist_loss, reg_loss) scalars.

Sharding: pure data parallel — batch b -> core b.

v6: all big matmul operands are fp8 (e4m3) with DoubleRow perf mode, which
sums two K-tile products per instruction at 0.5 PE-cycles per output column.
Hardware ISA constraints shaped the structure: DoubleRow LDWEIGHTS needs a
16-aligned pair step (one-hot padded 84->96 per tile), fp8 matmuls may only
write PSUM partition 0 (no tile_position quadrant packing -> per-Tt [32,512]
A/B accumulators + per-Tt stripe tails), GPSIMD cannot touch PSUM and only
runs Add/Multiply-class tensor ops (squares run on ACT directly or as a
DVE PSUM->bf16 copy + Pool bf16 multiply), and DVE reads PSUM at most once
per op.
  pass 1   segment sums via one-hot matmuls, DoubleRow over g-pairs.
  extract  sums -> block-diag(-mu) fp8 via bf16 selector matmuls and a
           host-precomputed -16/count vector (counts depend only on seg_gt,
           so they are host prep, like the one-hot layouts).
  pass 2   per-pixel D = e - mu via one fused (ident @ e + (-mu-table) @
           onehot) DoubleRow per 512-px tile; squares split ACT / DVE+Pool;
           hinge reduction via ones-block DoubleRow u-pairs; per-pixel w
           gather via a hi/lo two-term fp8 split of 256*w; sqrt/hinge/
           weight tail pipelined per Tt block at partitions 0..31.
One-hot tensors ship as fp8 from host (~same bytes as a u8 seg layout).
HBM traffic is ~7.9 MB/core.  The tiny 21x21 centroid pairwise loss and
final scalar assembly run on host from the per-core [84,128] segment-sum
matrix and [32,4] partial var sums.
"""

import os
import sys
from contextlib import ExitStack

import numpy as np

for _p in ("/opt/trn_rl_repo", "/root/.axon_site/_ro/trn_rl_repo"):
    if os.path.isdir(_p) and _p not in sys.path:
        sys.path.insert(0, _p)

import ml_dtypes

import concourse.bass as bass
import concourse.bacc as bacc
import concourse.tile as tile
from concourse import mybir
from concourse.bass_utils import run_bass_kernel_spmd

FP8 = ml_dtypes.float8_e4m3
BF16 = ml_dtypes.bfloat16

B, D, N = 8, 32, 65536
LP = 21          # label slots 0..20 (0 = background)
C = 4            # chunk count (channel-on-partition packing)
NC4 = N // C     # 16384 pixels per chunk
G = 128          # pass-1 tiles (512 px each)
A4 = 4           # pixels per partition per pass-1 tile
T2 = 32          # pass-2 tiles (512 cols each)
DELTA_V = 0.5
DELTA_D = 3.0
WSCALE = 256.0   # w gathered as WSCALE*w (fp8 range); host divides back
XSCALE = 16.0    # segment sums scaled by 1/XSCALE before fp8 (range fit)

F32 = mybir.dt.float32
F8 = mybir.dt.float8e4
BF = mybir.dt.bfloat16
OP = mybir.AluOpType
AF = mybir.ActivationFunctionType
DR = mybir.MatmulPerfMode.DoubleRow

# const tensor column offsets (fp8 [128, CST_W])
OFF_IOH = 0       # [128, 256]   slot0 identity, slot1 -mu (device-written)
OFF_ONES = 256    # [128, 8*32]  u-diagonal ones blocks
OFF_W8H = 512     # [128, 8*32]  u-diagonal WSCALE*w hi
OFF_W8L = 768     # [128, 8*32]  u-diagonal WSCALE*w lo
CST_W = 1024

# square path per D-pair (2 tiles): "A" = one ACT Square over the
# [128,1024] pair straight from PSUM; "C" = per-tile DVE copy PSUM->bf16
# SBUF + Pool bf16 multiply.  Last block all-ACT for the shortest
# post-DMA trail.
# per block of 8 tiles: how many go through the C path (DVE copy + Pool
# multiply); the rest are ACT Squares.  ACT tiles are emitted first within
# each block so the ACT queue is never starved behind the slower C path.
_C_PER_BLOCK = [4, 4, 4, 0]


def build_nc(stage=5, c_per_block=None, tail_delay=1, n_u4_slabs=16,
             c_first=False):
    c_per_block = c_per_block or _C_PER_BLOCK
    nc = bacc.Bacc()
    oht_d = nc.dram_tensor("oht", [128, G * 84], F8, kind="ExternalInput")
    selb_d = nc.dram_tensor("selb", [84, 84], BF, kind="ExternalInput")
    ebt_d = nc.dram_tensor("ebt", [128, G * 128], F8, kind="ExternalInput")
    u4_d = nc.dram_tensor("u4", [128, T2 * 1024], F8, kind="ExternalInput")
    cst8_d = nc.dram_tensor("cst8", [128, CST_W], F8, kind="ExternalInput")
    nrec_d = nc.dram_tensor("nrec", [128, 1], F32, kind="ExternalInput")
    xout_d = nc.dram_tensor("xout", [84, 128], F32, kind="ExternalOutput")
    vout_d = nc.dram_tensor("vout", [32, 4], F32, kind="ExternalOutput")

    with ExitStack() as ctx:
        tc = ctx.enter_context(tile.TileContext(nc))
        big = ctx.enter_context(tc.tile_pool(name="big", bufs=1))
        sm = ctx.enter_context(tc.tile_pool(name="sm", bufs=1))
        dbf = ctx.enter_context(tc.tile_pool(name="dbf", bufs=2))
        tlp = ctx.enter_context(tc.tile_pool(name="tlp", bufs=2))

        nrec = sm.tile([128, 1], F32)
        nc.sync.dma_start(out=nrec, in_=nrec_d[:, :])
        # parity-major layouts: pair (2gp, 2gp+1) sits at a fixed large
        # 16-aligned stride, satisfying the dual-fp8 LDWEIGHTS step rule
        # without padding
        OHT = big.tile([128, 2, G // 2, 84], F8)
        EBT = big.tile([128, 2, G // 2, 128], F8)
        ohtv = oht_d[:, :].rearrange("p (s g l) -> p s g l", s=2, g=G // 2)
        ebtv = ebt_d[:, :].rearrange("p (s g m) -> p s g m", s=2, g=G // 2)
        for i in range(4):
            g0, g1 = i * 16, (i + 1) * 16
            nc.sync.dma_start(out=OHT[:, :, g0:g1, :],
                              in_=ohtv[:, :, g0:g1, :])
            nc.sync.dma_start(out=EBT[:, :, g0:g1, :],
                              in_=ebtv[:, :, g0:g1, :])
        cst8 = sm.tile([128, CST_W], F8)
        nc.sync.dma_start(out=cst8, in_=cst8_d[:, :])
        selb = sm.tile([84, 84], BF)
        nc.sync.dma_start(out=selb, in_=selb_d[:, :])
        # pass-2 inputs: U4 slot t holds [emb4_t | oh4_t] fp8
        U4 = big.tile([128, T2, 2, 512], F8)
        for i in range(n_u4_slabs):
            w = T2 // n_u4_slabs
            t0, t1 = i * w, (i + 1) * w
            nc.sync.dma_start(out=U4[:, t0:t1, :, :],
                              in_=u4_d[:, t0 * 1024:t1 * 1024])

        IOH = cst8[:, OFF_IOH:OFF_IOH + 256].rearrange(
            "p (s m) -> p s m", s=2)
        ONES = cst8[:, OFF_ONES:OFF_ONES + 256].rearrange(
            "p (u k) -> p u k", u=8)
        W8H = cst8[:, OFF_W8H:OFF_W8H + 256].rearrange(
            "p (u k) -> p u k", u=8)
        W8L = cst8[:, OFF_W8L:OFF_W8L + 256].rearrange(
            "p (u k) -> p u k", u=8)
        SEL = selb[:, :].rearrange("p (a l) -> p a l", a=4)

        # prime the ACT table with the sqrt set (contains copy/square/sqrt)
        # so no reload lands on the critical path later
        prim = sm.tile([128, 1], F32)
        nc.scalar.activation(prim, nrec, AF.Sqrt, bias=0.0, scale=0.0)
        zbias = sm.tile([128, 1], F32)
        nc.scalar.activation(zbias, prim, AF.Copy, bias=0.0, scale=0.0)

        # ---- pass 1: X[(a,l), (a,d)], DoubleRow over g-pairs; the
        # one-hot is padded 84 -> 96 per g so the dual-fp8 LDWEIGHTS pair
        # step is 16-aligned (rows 84..95 of X are pad, ignored).  The
        # pass-1/extract PSUM pool closes before pass 2 so its two banks
        # recycle into the D pipeline. ----
        ps_ctx = ExitStack()
        ps = ps_ctx.enter_context(
            tc.tile_pool(name="ps", bufs=1, space="PSUM"))
        X_ps = ps.tile([84, 128], F32)
        for gp in range(G // 2):
            nc.tensor.matmul(
                X_ps,
                lhsT=OHT[:, :, gp, :],
                rhs=EBT[:, :, gp, :],
                start=(gp == 0), stop=(gp == G // 2 - 1), perf_mode=DR)
        Xs = sm.tile([84, 128], F32)
        nc.vector.tensor_copy(Xs, X_ps)
        nc.sync.dma_start(out=xout_d[:, :], in_=Xs)

        if stage >= 2:
            # ---- extract sums -> -means at the 4 partition blocks (bf16:
            # fp8 matmuls cannot target nonzero PSUM partition offsets) ----
            Xsb = sm.tile([84, 4, 32], BF)
            nc.scalar.activation(Xsb[:, :, :], X_ps, AF.Copy,
                                 bias=0.0, scale=1.0 / XSCALE)
            M_ps = ps.tile([128, 32], F32)
            for cb in range(4):
                tp = (0, cb * 32)
                for a in range(4):
                    nc.tensor.matmul(
                        M_ps[cb * 32:cb * 32 + 21, :],
                        lhsT=SEL[:, a, :],
                        rhs=Xsb[:, a, :],
                        start=(a == 0), stop=(a == 3), tile_position=tp,
                        skip_group_check=True)
            # selb rows carry -XSCALE/cnt, so M_ps is already -mean;
            # copy its diagonal blocks straight into IOH slot 1 (fp8),
            # split across DVE and ACT so the chain is two deep, not four
            for cb in range(4):
                sl = slice(cb * 32, cb * 32 + 21)
                dst = cst8[sl, OFF_IOH + 128 + cb * 32:
                           OFF_IOH + 128 + (cb + 1) * 32]
                if cb % 2 == 0:
                    nc.vector.tensor_copy(dst, M_ps[sl, :])
                else:
                    nc.scalar.activation(dst, M_ps[sl, :], AF.Copy,
                                         bias=0.0)
        ps_ctx.close()
        psD = ctx.enter_context(tc.tile_pool(name="psD", bufs=4, space="PSUM"))
        psA = ctx.enter_context(tc.tile_pool(name="psA", bufs=2, space="PSUM"))
        psB = ctx.enter_context(tc.tile_pool(name="psB", bufs=2, space="PSUM"))

        if stage >= 3:
            # ---- pass 2: per tile D build + square; per Tt block the B/A
            # DoubleRow reductions land at PSUM partition 0 and the scalar
            # tail consumes the block as a partition 0..31 stripe ----
            SQT = big.tile([128, T2, 512], F8)
            vn4 = sm.tile([32, 4], F32)
            AB = [None] * 4

            def emit_AB(Tt):
                B_ps = psB.tile([32, 512], F32)   # WSCALE*w per pixel
                A_ps = psA.tile([32, 512], F32)   # |e - mu|^2 per pixel
                if stage >= 4:
                    for hl, W8 in ((0, W8H), (1, W8L)):
                        for up in range(4):
                            u0 = 2 * up
                            nc.tensor.matmul(
                                B_ps,
                                lhsT=W8[:, u0:u0 + 2, :],
                                rhs=U4[:, Tt * 8 + u0:Tt * 8 + u0 + 2, 1, :],
                                start=(hl == 0 and up == 0),
                                stop=(hl == 1 and up == 3), perf_mode=DR)
                for up in range(4):
                    u0 = 2 * up
                    nc.tensor.matmul(
                        A_ps,
                        lhsT=ONES[:, u0:u0 + 2, :],
                        rhs=SQT[:, Tt * 8 + u0:Tt * 8 + u0 + 2, :],
                        start=(up == 0), stop=(up == 3), perf_mode=DR)
                AB[Tt] = (A_ps, B_ps)

            def emit_tail(Tt):
                # stripe tail at partitions 0..31: d = sqrt(A);
                # r = max(d - dv, 0); vn4[:, Tt] = sum(r*r*B)
                A_ps, B_ps = AB[Tt]
                d_sb = tlp.tile([32, 512], BF)
                nc.scalar.activation(d_sb, A_ps, AF.Sqrt,
                                     bias=zbias[0:32, 0:1])
                r_sb = tlp.tile([32, 512], BF)
                nc.vector.tensor_scalar(out=r_sb, in0=d_sb,
                                        scalar1=-DELTA_V, scalar2=0.0,
                                        op0=OP.add, op1=OP.max)
                r2_sb = tlp.tile([32, 512], BF)
                nc.vector.tensor_tensor(out=r2_sb, in0=r_sb, in1=r_sb,
                                        op=OP.mult)
                vw = tlp.tile([32, 512], F32)
                nc.vector.scalar_tensor_tensor(
                    out=vw, in0=r2_sb, scalar=0.0, in1=B_ps,
                    op0=OP.add, op1=OP.mult,
                    accum_out=vn4[:, Tt:Tt + 1])

            for Tt in range(4):
                ncb = c_per_block[Tt]
                if c_first:
                    order = [u for u in range(8) if u < ncb] + \
                            [u for u in range(8) if u >= ncb]
                else:
                    order = [u for u in range(8) if u >= ncb] + \
                            [u for u in range(8) if u < ncb]
                for u in order:
                    t = Tt * 8 + u
                    D_ps = psD.tile([128, 512], F32)
                    nc.tensor.matmul(D_ps, lhsT=IOH[:, :, :],
                                     rhs=U4[:, t, :, :],
                                     start=True, stop=True, perf_mode=DR)
                    if u < ncb:
                        Db = dbf.tile([128, 512], BF)
                        nc.vector.tensor_copy(Db, D_ps)
                        nc.gpsimd.tensor_tensor(
                            out=SQT[:, t, :], in0=Db, in1=Db, op=OP.mult)
                    else:
                        nc.scalar.activation(SQT[:, t, :], D_ps, AF.Square,
                                             bias=zbias[:, 0:1])
                if Tt > 0:
                    emit_AB(Tt - 1)
                if Tt >= tail_delay:
                    emit_tail(Tt - tail_delay)
            emit_AB(3)
            for Tt in range(4 - tail_delay, 4):
                emit_tail(Tt)
            nc.scalar.dma_start(out=vout_d[:, :], in_=vn4)
        else:
            vz = sm.tile([32, 4], F32)
            nc.vector.memset(vz, 0.0)
            nc.sync.dma_start(out=vout_d[:, :], in_=vz)

    nc.compile()
    return nc


def _make_consts():
    cst = np.zeros((128, CST_W), np.float32)
    cst[:, OFF_IOH:OFF_IOH + 128] = np.eye(128)
    ones8 = np.zeros((128, 8, 32), np.float32)
    for c in range(C):
        for d in range(32):
            for u in range(8):
                ones8[c * 32 + d, u, u * 4 + c] = 1.0
    cst[:, OFF_ONES:OFF_ONES + 256] = ones8.reshape(128, 256)
    return cst


def _prep_core(emb_b, seg_b, cst_base):
    """emb_b [32, 65536] f32, seg_b [65536] i32 -> per-core input map."""
    Tm = np.ascontiguousarray(emb_b.T)                       # [N, 32]
    t4 = Tm.reshape(G, 128, A4, 32).transpose(1, 0, 2, 3)    # [p, g, a, d]
    ebg = t4.reshape(128, G, 128)
    ebt = np.ascontiguousarray(
        ebg.reshape(128, G // 2, 2, 128).transpose(0, 2, 1, 3)).astype(FP8)
    ebt = ebt.reshape(128, G * 128)
    s4 = seg_b.reshape(G, 128, A4).transpose(1, 0, 2)        # [p, g, a]
    ohg = (s4[:, :, :, None] == np.arange(LP)[None, None, None, :]).reshape(
        128, G, 84)
    oht = np.ascontiguousarray(
        ohg.reshape(128, G // 2, 2, 84).transpose(0, 2, 1, 3)).astype(FP8)
    emb4 = np.ascontiguousarray(
        emb_b.reshape(32, C, NC4).transpose(1, 0, 2)).reshape(128, NC4)
    oh4 = (seg_b.reshape(C, 1, NC4)
           == np.arange(32)[None, :, None]).reshape(128, NC4)
    u4 = np.empty((128, T2, 2, 512), FP8)
    u4[:, :, 0, :] = emb4.astype(FP8).reshape(128, T2, 512)
    u4[:, :, 1, :] = oh4.astype(FP8).reshape(128, T2, 512)

    # host-side seg statistics: counts -> -XSCALE/cnt and the WSCALE*w
    # hi/lo fp8 split, in the (c,l)-partition / u-diagonal layouts
    counts = np.bincount(seg_b, minlength=LP)[:LP].astype(np.float64)
    nrec = np.zeros((128, 1), np.float32)
    w = np.zeros(LP, np.float64)
    w[1:] = (counts[1:] > 0) / np.maximum(counts[1:], 1.0)
    for c in range(C):
        nrec[c * 32:c * 32 + LP, 0] = -XSCALE / np.maximum(counts, 1.0)
    ws = (w * WSCALE).astype(np.float32)
    whi = ws.astype(FP8)
    wlo = (ws - whi.astype(np.float32)).astype(FP8)
    cst = cst_base.copy()
    for c in range(C):
        for u in range(8):
            cst[c * 32:c * 32 + LP, OFF_W8H + u * 32 + u * 4 + c] = whi
            cst[c * 32:c * 32 + LP, OFF_W8L + u * 32 + u * 4 + c] = wlo
    selv = np.zeros((84, 84), np.float32)
    for a in range(A4):
        for l in range(LP):
            selv[a * 21 + l, a * 21 + l] = -XSCALE / max(counts[l], 1.0)
    return {
        "oht": oht.reshape(128, G * 84),
        "selb": selv.astype(BF16),
        "ebt": ebt,
        "u4": u4.reshape(128, T2 * 1024),
        "cst8": cst.astype(FP8),
        "nrec": nrec,
    }


_NC_CACHE = None


def _get_nc():
    global _NC_CACHE
    if _NC_CACHE is None:
        _NC_CACHE = build_nc()
    return _NC_CACHE


def _host_finish(X, vn, seg_b):
    """X [84, 128] f32 (pass-1 sums), vn [32, 4] f32 -> (var_b, dist_b)."""
    counts = np.bincount(seg_b, minlength=LP)[:LP].astype(np.float64)
    Xr = X.reshape(A4, LP, 128).astype(np.float64)
    sums = np.zeros((LP, 32))
    for a in range(A4):
        sums += Xr[a, :, a * 32:(a + 1) * 32]
    means = sums / np.maximum(counts, 1.0)[:, None]
    pres = counts > 0
    pres[0] = False
    nl = float(pres.sum())
    var_b = float(vn.sum()) / WSCALE / max(nl, 1.0) if nl > 0 else 0.0
    m = means[1:]
    p = pres[1:]
    sqd = ((m[:, None, :] - m[None, :, :]) ** 2).sum(-1)
    dist = np.sqrt(np.maximum(sqd, 0.0))
    pair = (p[:, None] & p[None, :]) & ~np.eye(LP - 1, dtype=bool)
    dl = (np.maximum(DELTA_D - dist, 0.0) ** 2 * pair).sum()
    denom = max(nl * (nl - 1.0), 1.0)
    dist_b = dl / denom / 2.0 if nl > 1 else 0.0
    return var_b, dist_b


def kernel(embedding, seg_gt):
    embedding = np.asarray(embedding, np.float32)
    seg_gt = np.asarray(seg_gt, np.int32)
    cst_base = _make_consts()
    in_maps = [_prep_core(embedding[b], seg_gt[b], cst_base)
               for b in range(B)]
    nc = _get_nc()
    res = run_bass_kernel_spmd(nc, in_maps, core_ids=list(range(B)))
    var_l, dist_l = [], []
    for b in range(B):
        var_b, dist_b = _host_finish(res.results[b]["xout"],
                                     res.results[b]["vout"], seg_gt[b])
        var_l.append(var_b)
        dist_l.append(dist_b)
    return (np.float32(np.mean(var_l)), np.float32(np.mean(dist_l)),
            np.float32(0.0))
